# revision 16
# baseline (speedup 1.0000x reference)
"""MinGRU Trainium2 kernel.

Reference computation (per batch b):
    c = depthwise_conv1d(x, conv_w, taps=5, pad=2)        # [D, L]
    h = h_w @ c                                           # [O, L]
    g = concat([-1000, +1000], g_w @ c)                   # [O, L]
    a = sigmoid(-g); v = sigmoid(g) * h
    out[l] = a[l] * out[l-1] + v[l]     (linear scan along L)

Strategy: pure data-parallel over B (8 batches -> 8 NeuronCores).
Per core everything streams in l-chunks of 512:
  - conv: 5 diagonal-matmuls on TensorE accumulating in PSUM
  - c PSUM->SBUF copy on ScalarE
  - h/g 1x1-conv matmuls on TensorE (float32r, 1 cyc/row)
  - a = sigmoid(-(g+bias)) on ScalarE (bias carries the +/-1000 polarized rows)
  - z = 1 - a on VectorE (tensor_scalar 2x mode)
  - v = z*h on VectorE, scan via tensor_tensor_scan on VectorE
  - channel 1 replicates the reference's f32 log-domain quantization:
    out[1,l] = sign(h) * exp(fl32(fl32(K_l + fl32(ln|h|)) - K_l)), K_l = 1000(l+1)
  - channel 0 output is exactly 0 (v row forced to 0, a=1).
"""

import numpy as np

import concourse.bass as bass
import concourse.mybir as mybir
from concourse import bacc
from concourse.tile import TileContext
from concourse.bass_utils import run_bass_kernel_spmd

F32 = mybir.dt.float32
F32R = mybir.dt.float32r
AF = mybir.ActivationFunctionType
OP = mybir.AluOpType

B, D, O, L = 8, 512, 512, 4096
P = 128
CH = 512                 # l-chunk width (one PSUM bank)
NCH = L // CH            # 8
NDT = D // P             # 4 d-tiles
NOT = O // P             # 4 o-tiles
NTAPS = 5
N_CORES = 8


def build_program():
    nc = bacc.Bacc()

    x = nc.declare_dram_parameter("x", [D, L], F32R, isOutput=False)
    hwT = nc.declare_dram_parameter("hwT", [D, O], F32R, isOutput=False)
    gwT = nc.declare_dram_parameter("gwT", [D, O], F32R, isOutput=False)
    cwdiag = nc.declare_dram_parameter("cwdiag", [D, NTAPS * P], F32R, isOutput=False)
    gbp = nc.declare_dram_parameter("gbp", [O, 1], F32, isOutput=False)
    gbn = nc.declare_dram_parameter("gbn", [O, 1], F32, isOutput=False)
    krow = nc.declare_dram_parameter("krow", [2, L], F32, isOutput=False)
    zpad = nc.declare_dram_parameter("zpad", [P, 2], F32R, isOutput=False)
    out = nc.declare_dram_parameter("out", [O, L], F32, isOutput=True)

    with TileContext(nc) as tc:
        with (
            tc.tile_pool(name="weights", bufs=1) as wpool,
            tc.tile_pool(name="xin", bufs=3) as xpool,
            tc.tile_pool(name="csb", bufs=6) as cpool,
            tc.tile_pool(name="actout", bufs=3) as apool,
            tc.tile_pool(name="vtiles", bufs=3) as vpool,
            tc.tile_pool(name="outt", bufs=12) as opool,
            tc.tile_pool(name="ch1", bufs=3) as ch1pool,
            tc.tile_pool(name="cps", bufs=2, space="PSUM") as cps_pool,
            tc.tile_pool(name="hps", bufs=2, space="PSUM") as hps_pool,
            tc.tile_pool(name="gps", bufs=2, space="PSUM") as gps_pool,
        ):
            # ---- persistent weights ----
            hwT_sb, gwT_sb, cw_sb, gbp_sb, gbn_sb = [], [], [], [], []
            for dt in range(NDT):
                t = wpool.tile([P, O], F32R, tag=f"hwT{dt}")
                nc.sync.dma_start(out=t, in_=hwT[dt * P:(dt + 1) * P, :])
                hwT_sb.append(t)
                t = wpool.tile([P, O], F32R, tag=f"gwT{dt}")
                nc.sync.dma_start(out=t, in_=gwT[dt * P:(dt + 1) * P, :])
                gwT_sb.append(t)
                t = wpool.tile([P, NTAPS * P], F32R, tag=f"cw{dt}")
                nc.sync.dma_start(out=t, in_=cwdiag[dt * P:(dt + 1) * P, :])
                cw_sb.append(t)
            for ot in range(NOT):
                t = wpool.tile([P, 1], F32, tag=f"gbp{ot}")
                nc.sync.dma_start(out=t, in_=gbp[ot * P:(ot + 1) * P, :])
                gbp_sb.append(t)
                t = wpool.tile([P, 1], F32, tag=f"gbn{ot}")
                nc.sync.dma_start(out=t, in_=gbn[ot * P:(ot + 1) * P, :])
                gbn_sb.append(t)
            krow_sb = wpool.tile([2, L], F32, tag="krow")
            nc.sync.dma_start(out=krow_sb, in_=krow[:, :])

            # software-pipelined over chunks: conv(i) emitted before h/g(i-1)
            c_sb = [None] * (NCH)        # [chunk] -> list of 4 SBUF c tiles
            prev_out = [None] * NOT      # previous chunk's out tiles per o-tile

            def emit_conv(i):
                lo = i * CH
                tiles = []
                for dt in range(NDT):
                    # xt covers x columns [lo-2, lo+CH+2); halo columns that
                    # fall outside [0, L) are zero-filled via a tiny DMA from
                    # the zpad constant (keeps every producer fp32r-typed).
                    xt = xpool.tile([P, CH + 4], F32R, tag="xt")
                    if i == 0:
                        nc.sync.dma_start(out=xt[:, 0:2], in_=zpad[:, :])
                        nc.sync.dma_start(out=xt[:, 2:CH + 4],
                                          in_=x[dt * P:(dt + 1) * P, 0:CH + 2])
                    elif i == NCH - 1:
                        nc.sync.dma_start(out=xt[:, CH + 2:CH + 4], in_=zpad[:, :])
                        nc.sync.dma_start(out=xt[:, 0:CH + 2],
                                          in_=x[dt * P:(dt + 1) * P, lo - 2:lo + CH])
                    else:
                        nc.sync.dma_start(out=xt[:, :],
                                          in_=x[dt * P:(dt + 1) * P, lo - 2:lo + CH + 2])
                    cp = cps_pool.tile([P, CH], F32, tag="cps")
                    for k in range(NTAPS):
                        nc.tensor.matmul(
                            cp,
                            lhsT=cw_sb[dt][:, k * P:(k + 1) * P],
                            rhs=xt[:, k:k + CH],
                            start=(k == 0), stop=(k == NTAPS - 1),
                        )
                    ct = cpool.tile([P, CH], F32R, tag="ct")
                    nc.scalar.copy(ct, cp)
                    tiles.append(ct)
                c_sb[i] = tiles

            def emit_rest(i):
                lo = i * CH
                for ot in range(NOT):
                    hp = hps_pool.tile([P, CH], F32, tag="hps")
                    for dt in range(NDT):
                        nc.tensor.matmul(
                            hp,
                            lhsT=hwT_sb[dt][:, ot * P:(ot + 1) * P],
                            rhs=c_sb[i][dt],
                            start=(dt == 0), stop=(dt == NDT - 1),
                        )
                    gp = gps_pool.tile([P, CH], F32, tag="gps")
                    for dt in range(NDT):
                        nc.tensor.matmul(
                            gp,
                            lhsT=gwT_sb[dt][:, ot * P:(ot + 1) * P],
                            rhs=c_sb[i][dt],
                            start=(dt == 0), stop=(dt == NDT - 1),
                        )
                    # a = sigmoid(-(g + bias)) ; z = 1 - a ; v = z * h
                    at = apool.tile([P, CH], F32, tag="at")
                    nc.scalar.activation(at, gp, AF.Sigmoid, bias=gbn_sb[ot], scale=-1.0)
                    zt = vpool.tile([P, CH], F32, tag="zt")
                    nc.vector.tensor_scalar(zt, at, -1.0, 1.0, OP.mult, OP.add)
                    vt = vpool.tile([P, CH], F32, tag="vt")
                    nc.vector.tensor_tensor(vt, zt, hp, OP.mult)
                    ott = opool.tile([P, CH], F32, tag=f"out{ot}")
                    init = 0.0 if i == 0 else prev_out[ot][:, CH - 1:CH]
                    nc.vector.tensor_tensor_scan(ott, at, vt, init, OP.mult, OP.add)
                    if ot == 0:
                        # channels 0/1 computed on rows 0:2 (partition slices
                        # must start at 0), then row 0 is forced back to zero.
                        # channel 1 replicates the reference's log-domain
                        # f32 quantization: sign(h)*exp(fl(fl(K+ln|h|)-K)).
                        habs = ch1pool.tile([2, CH], F32, tag="habs")
                        nc.scalar.activation(habs, hp[0:2, :], AF.Abs)
                        hsign = ch1pool.tile([2, CH], F32, tag="hsign")
                        nc.scalar.activation(hsign, hp[0:2, :], AF.Sign)
                        lam = ch1pool.tile([2, CH], F32, tag="lam")
                        nc.vector.tensor_scalar_max(lam, habs, 1e-6)
                        nc.scalar.activation(lam, lam, AF.Ln)
                        kseg = krow_sb[0:2, lo:lo + CH]
                        nc.vector.tensor_tensor(lam, lam, kseg, OP.add)
                        nc.vector.tensor_tensor(lam, lam, kseg, OP.subtract)
                        nc.scalar.activation(lam, lam, AF.Exp)
                        nc.vector.tensor_tensor(ott[0:2, :], lam, hsign, OP.mult)
                        nc.gpsimd.memset(ott[0:1, :], 0.0)
                    nc.sync.dma_start(out=out[ot * P:(ot + 1) * P, lo:lo + CH], in_=ott)
                    prev_out[ot] = ott

            emit_conv(0)
            for i in range(1, NCH):
                emit_conv(i)
                emit_rest(i - 1)
            emit_rest(NCH - 1)

    nc.finalize()
    return nc


_PROGRAM = None


def _get_program():
    global _PROGRAM
    if _PROGRAM is None:
        _PROGRAM = build_program()
    return _PROGRAM


def prepare_in_maps(x, conv_w, h_w, g_w):
    x = np.ascontiguousarray(np.asarray(x), dtype=np.float32)
    conv_w = np.asarray(conv_w, dtype=np.float32)
    h_w = np.asarray(h_w, dtype=np.float32)
    g_w = np.asarray(g_w, dtype=np.float32)

    hwT = np.ascontiguousarray(h_w[:, :, 0].T)                    # [D, O]
    gw_pad = np.zeros((O, D), np.float32)
    gw_pad[2:, :] = g_w[:, :, 0]
    gwT = np.ascontiguousarray(gw_pad.T)                          # [D, O]

    # 5 diagonal matrices per d-tile, concatenated along free dim: [D, 5*128]
    cwdiag = np.zeros((D, NTAPS * P), np.float32)
    for dt in range(NDT):
        for k in range(NTAPS):
            blk = cwdiag[dt * P:(dt + 1) * P, k * P:(k + 1) * P]
            np.fill_diagonal(blk, conv_w[dt * P:(dt + 1) * P, 0, k])

    gbp = np.zeros((O, 1), np.float32)
    gbp[0, 0], gbp[1, 0] = -1000.0, 1000.0
    gbn = -gbp
    krow = (1000.0 * (np.arange(L, dtype=np.float64) + 1.0)).astype(np.float32)
    krow = np.ascontiguousarray(np.broadcast_to(krow.reshape(1, L), (2, L)))

    zpad = np.zeros((P, 2), np.float32)
    return [
        {"x": x[b], "hwT": hwT, "gwT": gwT, "cwdiag": cwdiag,
         "gbp": gbp, "gbn": gbn, "krow": krow, "zpad": zpad}
        for b in range(B)
    ]


def kernel(x, conv_w, h_w, g_w):
    in_maps = prepare_in_maps(x, conv_w, h_w, g_w)
    nc = _get_program()
    res = run_bass_kernel_spmd(nc, in_maps, list(range(N_CORES))).results
    return np.stack([res[b]["out"] for b in range(B)], axis=0)
